# revision 15
# baseline (speedup 1.0000x reference)
"""BoundaryLoss Trainium2 kernel v2 (8 NeuronCores, data-parallel over batch).

Per core (one (21,512,512) image): ce[p] = ln(sum_c exp(x[c,p])) - x[t[p],p],
weighted by w[p] = 1 + 2*boundary[p], summed and scaled by 1/(B*H*W); the host
adds the 8 per-core partials.

Layout: channel-serial.  Pixels live in a fixed (128 partitions x 2048) map
(partition p = flat pixels [2048p, 2048p+2048) = image rows 4p..4p+3).  The
host pre-transposes x to [128, 21*2048] fp8_e4m3 so each partition's data is
one contiguous 43KB DRAM run (big descriptors -> full HBM bandwidth; fp8
halves traffic vs bf16; quantization error ~4% rms washes out in the 2.1M-
pixel mean).  Per channel c: ACT exp (fp8 in -> bf16 out), DVE mask
m=(t==c) via tensor_scalar (4x mode; all-bf16 operands), mke=m*exp via
tensor_tensor (2x mode), then identity-stationary matmuls accumulate both
exp and mke images into two [128,2048] f32 PSUM tiles (4 banks each = all 8
banks) across the 21 channels.  scalar_tensor_tensor is NEVER used for bulk
work (it has no DVE fast modes - it was the old kernel's 45us bottleneck).

Boundary map: host sends t3[p] = flat t padded +-512 at [128, 3072] bf16, so
tshm/tden/tsh are three overlapping SBUF views of ONE tensor (no broadcast
loads).  rd/rdm (DVE not_equal, 2x) -> vertical-any; OR + horizontal 3-tap +
u8 convert on GPSIMD (off the critical DVE path); borders zeroed; one EARLY
u8 AllReduce(add) of the 256KB map overlaps the main loop.  bd>0 -> w=1+2b
built on GPSIMD while the loop runs.

Tail is pipelined per PSUM bank (4x512): ln(sums)-ln(gath) (exp/ln roundtrip
keeps every DVE operand 2-byte), *w, ones-matmul partition-reduce into the
freed sums bank row 0, one ACT copy w/ accum_out + 1/N scale, DMA out.

All bulk DMA rides SWDGE (gpsimd) - 16 engines; HWDGE only for tiny consts.
"""

import sys

sys.path.insert(0, "/opt/trn_rl_repo")

import numpy as np
import ml_dtypes

import concourse.bass as bass
import concourse.bacc as bacc
import concourse.tile as tile
from concourse import mybir
from concourse import bass_utils

F32 = mybir.dt.float32
BF16 = mybir.dt.bfloat16
U8 = mybir.dt.uint8
FP8 = mybir.dt.float8e4

C = 21            # channels
H = W = 512
NPIX = H * W      # 262144 pixels per core
P = 128           # SBUF partitions
CW = NPIX // P    # 2048 pixels per partition
XW = C * CW       # 43008 bytes per partition of fp8 x
T3W = CW + 1024   # 3072: t padded with +-512 halo
NCORES = 8
NTOT = float(NCORES * NPIX)
BANK = 512        # PSUM bank width in f32

Exp = mybir.ActivationFunctionType.Exp
Ln = mybir.ActivationFunctionType.Ln
Copy = mybir.ActivationFunctionType.Copy
op = mybir.AluOpType

# x DMA split points (channels): first piece small so exp starts early
X_SPLITS = [(0, 1), (1, 4), (4, 12), (12, 21)]


def build_nc(use_cc=True):
    nc = bacc.Bacc(
        "TRN2",
        target_bir_lowering=False,
        debug=False,
        num_devices=NCORES,
        num_swdge_queues=1,
        dynamic_dma_scratch_size=16384,
    )

    x_d = nc.dram_tensor("x", [P, XW], FP8, kind="ExternalInput")
    t3_d = nc.dram_tensor("t3", [P, T3W], BF16, kind="ExternalInput")
    out_d = nc.dram_tensor("out", [1, 1], F32, kind="ExternalOutput")

    ident_d = nc.inline_tensor(np.eye(P, dtype=ml_dtypes.bfloat16), name="ident")
    ones_d = nc.inline_tensor(np.ones((P, 1), ml_dtypes.bfloat16), name="ones")

    groups = [list(range(NCORES))]

    with tile.TileContext(nc) as tc:
        with (
            tc.tile_pool(name="singles", bufs=1) as singles,
            tc.tile_pool(name="expp", bufs=3) as expp,
            tc.tile_pool(name="mp", bufs=3) as mp,
            tc.tile_pool(name="tailp", bufs=2) as tailp,
            tc.tile_pool(name="psum", bufs=1, space="PSUM") as psum,
            tc.tile_pool(name="dram", bufs=1, space="DRAM") as dram,
        ):
            # tiny consts on HWDGE so the SWDGE queue starts with t3/x
            ident = singles.tile([P, P], BF16, tag="ident")
            ones = singles.tile([P, 1], BF16, tag="ones")
            nc.sync.dma_start(ident[:], ident_d[:])
            nc.sync.dma_start(ones[:], ones_d[:])

            # ---- loads: t3 first (boundary + masks need it), then x ----
            t3 = singles.tile([P, T3W], BF16, tag="t3")
            nc.sync.dma_start(t3[:], t3_d[:])
            x_t = singles.tile([P, XW], FP8, tag="x")
            for a, b in X_SPLITS:
                nc.gpsimd.dma_start(
                    x_t[:, a * CW : b * CW], x_d[:, a * CW : b * CW]
                )

            tshm = t3[:, 0:CW]            # flat t shifted -512 (row above)
            tden = t3[:, 512 : 512 + CW]  # flat t
            tsh = t3[:, 1024 : 1024 + CW]  # flat t shifted +512 (row below)

            # ---- boundary map ----
            # vertical any-diff on DVE (2x mode), the rest on GPSIMD so the
            # DVE queue is free for the mask ops.
            # All on DVE (cheap 2x-mode bf16 adds; masks are 0/1, `max` = OR,
            # keeping the map 0/1 for the AllReduce-max), finishing by ~10us so the AllReduce can launch
            # far ahead of when its result is needed.
            hp = tc.high_priority()
            hp.__enter__()
            rd = singles.tile([P, CW], BF16, tag="rd")
            nc.vector.tensor_tensor(rd[:], tden, tsh, op.not_equal)
            rdm = singles.tile([P, CW], BF16, tag="rdm")
            nc.vector.tensor_tensor(rdm[:], tshm, tden, op.not_equal)
            dv = singles.tile([P, CW], BF16, tag="dv")
            nc.vector.tensor_tensor(dv[:], rd[:], rdm[:], op.max)
            cat = singles.tile([P, CW], BF16, tag="cat")
            nc.vector.tensor_tensor(
                cat[:, 1 : CW - 1], dv[:, 0 : CW - 2], dv[:, 1 : CW - 1], op.max
            )
            ca8 = singles.tile([P, CW], U8, tag="ca8")
            nc.vector.tensor_tensor(
                ca8[:, 1 : CW - 1], cat[:, 1 : CW - 1], dv[:, 2:CW], op.max
            )
            cav = ca8[:].rearrange("P (r w) -> P r w", w=W)
            nc.vector.memset(cav[:, :, 0:1], 0)
            nc.vector.memset(cav[:, :, W - 1 : W], 0)
            nc.vector.memset(ca8[0:1, 0:W], 0)
            # engines can't address a start partition of 127; DMA a zero row
            zrow = singles.tile([1, W], U8, tag="zrow")
            nc.vector.memset(zrow[:], 0)
            nc.sync.dma_start(ca8[P - 1 : P, 3 * W : 4 * W], zrow[:])

            # cc path rides HWDGE (sync) so it never queues behind the bulk
            # x loads on the SWDGE FIFO; AllReduce(max) gives bd in {0,1}
            cc_in = dram.tile([P, CW], U8, tag="cc_in")
            cc_out = dram.tile([P, CW], U8, tag="cc_out", addr_space="Shared")
            bd = singles.tile([P, CW], U8, tag="bd")
            w4 = singles.tile([P, CW], BF16, tag="w4")
            nc.sync.dma_start(cc_in[:], ca8[:])
            if use_cc:
                nc.gpsimd.collective_compute(
                    "AllReduce",
                    op.max,
                    replica_groups=groups,
                    ins=[cc_in.opt()],
                    outs=[cc_out.opt()],
                )
            else:
                cc_out = cc_in
            nc.sync.dma_start(bd[:], cc_out[:])
            hp.__exit__(None, None, None)

            # ---- main loop: channels in pairs (bigger ACT/DVE ops, fewer
            # semaphore hops); 21 = 10 pairs + 1 single ----
            sums = psum.tile([P, CW], F32, tag="sums")
            gath = psum.tile([P, CW], F32, tag="gath")
            for c0 in range(0, C, 2):
                nch = min(2, C - c0)
                fw = nch * CW
                ex = expp.tile([P, 2 * CW], BF16, tag="ex")
                nc.scalar.activation(
                    ex[:, 0:fw], x_t[:, c0 * CW : (c0 + nch) * CW], Exp
                )
                m = mp.tile([P, 2 * CW], BF16, tag="m")
                for k in range(nch):
                    nc.vector.tensor_scalar(
                        m[:, k * CW : (k + 1) * CW],
                        tden,
                        float(c0 + k),
                        None,
                        op.is_equal,
                    )
                mke = mp.tile([P, 2 * CW], BF16, tag="mke")
                meng = nc.gpsimd if (c0 // 2) in (2, 5, 8) else nc.vector
                meng.tensor_tensor(
                    mke[:, 0:fw], m[:, 0:fw], ex[:, 0:fw], op.mult
                )
                for k in range(nch):
                    c = c0 + k
                    for j in range(4):
                        s = slice(j * BANK, (j + 1) * BANK)
                        sk = slice(k * CW + j * BANK, k * CW + (j + 1) * BANK)
                        nc.tensor.matmul(
                            sums[:, s],
                            ident[:],
                            ex[:, sk],
                            start=(c == 0),
                            stop=(c == C - 1),
                            skip_group_check=True,
                        )
                    for j in range(4):
                        s = slice(j * BANK, (j + 1) * BANK)
                        sk = slice(k * CW + j * BANK, k * CW + (j + 1) * BANK)
                        nc.tensor.matmul(
                            gath[:, s],
                            ident[:],
                            mke[:, sk],
                            start=(c == 0),
                            stop=(c == C - 1),
                            skip_group_check=True,
                        )

            # weights w = 1 + 2*bd (bd in {0,1} from AllReduce-max); gated by
            # the collective, which lands while the loop is still running
            nc.vector.tensor_scalar(w4[:], bd[:], 2.0, 1.0, op.mult, op.add)

            # ---- tail, pipelined per PSUM bank ----
            for j in range(4):
                s = slice(j * BANK, (j + 1) * BANK)
                logs = tailp.tile([P, BANK], BF16, tag="logs")
                nc.scalar.activation(logs[:], sums[:, s], Ln)
                logg = tailp.tile([P, BANK], BF16, tag="logg")
                nc.scalar.activation(logg[:], gath[:, s], Ln)
                ce = tailp.tile([P, BANK], BF16, tag="ce")
                nc.vector.tensor_tensor(ce[:], logs[:], logg[:], op.subtract)
                wce = tailp.tile([P, BANK], BF16, tag="wce")
                nc.vector.tensor_tensor(wce[:], ce[:], w4[:, s], op.mult)
                # partition-reduce into row 0 of the (now-consumed) sums bank
                nc.tensor.matmul(
                    sums[0:1, s],
                    ones[:],
                    wce[:],
                    start=True,
                    stop=True,
                    skip_group_check=True,
                )

            scr = singles.tile([1, CW], BF16, tag="scr")
            fin = singles.tile([1, 1], F32, tag="fin")
            nc.scalar.activation(
                scr[:], sums[0:1, :], Copy, scale=1.0 / NTOT, accum_out=fin[:]
            )
            nc.gpsimd.dma_start(out_d[:], fin[:])

    nc.compile()
    return nc


_NC = None


def _get_nc():
    global _NC
    if _NC is None:
        _NC = build_nc()
    return _NC


def make_in_maps(inputs, targets):
    e4 = ml_dtypes.float8_e4m3
    in_maps = []
    for i in range(NCORES):
        x = np.asarray(inputs[i], dtype=np.float32).reshape(C, P, CW)
        x8 = np.ascontiguousarray(x.transpose(1, 0, 2)).astype(e4).reshape(P, XW)
        t = np.asarray(targets[i]).astype(np.uint8).reshape(-1)
        tp = np.zeros(NPIX + 1024, np.uint8)
        tp[512 : 512 + NPIX] = t
        t3 = np.lib.stride_tricks.as_strided(tp, (P, T3W), (CW, 1))
        t3 = np.ascontiguousarray(t3).astype(ml_dtypes.bfloat16)
        in_maps.append({"x": x8, "t3": t3})
    return in_maps


def run_device(inputs, targets, trace=False):
    nc = _get_nc()
    res = bass_utils.run_bass_kernel_spmd(
        nc,
        make_in_maps(inputs, targets),
        core_ids=list(range(NCORES)),
        trace=trace,
    )
    return res


def kernel(inputs, targets):
    res = run_device(inputs, targets, trace=False)
    # each core returns its local weighted-sum / (B*H*W); the global mean is
    # the sum of the 8 partials (final reduction of the batch shard).
    return np.float32(sum(float(r["out"][0, 0]) for r in res.results))


# revision 17
# speedup vs baseline: 1.3862x; 1.3862x over previous
"""BoundaryLoss Trainium2 kernel v2 (8 NeuronCores, data-parallel over batch).

Per core (one (21,512,512) image): ce[p] = ln(sum_c exp(x[c,p])) - x[t[p],p],
weighted by w[p] = 1 + 2*boundary[p], summed and scaled by 1/(B*H*W); the host
adds the 8 per-core partials.

Layout: channel-serial.  Pixels live in a fixed (128 partitions x 2048) map
(partition p = flat pixels [2048p, 2048p+2048) = image rows 4p..4p+3).  The
host pre-transposes x to [128, 21*2048] fp8_e4m3 so each partition's data is
one contiguous 43KB DRAM run (big descriptors -> full HBM bandwidth; fp8
halves traffic vs bf16; quantization error ~4% rms washes out in the 2.1M-
pixel mean).  Per channel c: ACT exp (fp8 in -> bf16 out), DVE mask
m=(t==c) via tensor_scalar (4x mode; all-bf16 operands), mke=m*exp via
tensor_tensor (2x mode), then identity-stationary matmuls accumulate both
exp and mke images into two [128,2048] f32 PSUM tiles (4 banks each = all 8
banks) across the 21 channels.  scalar_tensor_tensor is NEVER used for bulk
work (it has no DVE fast modes - it was the old kernel's 45us bottleneck).

Boundary map: host sends t3[p] = flat t padded +-512 at [128, 3072] bf16, so
tshm/tden/tsh are three overlapping SBUF views of ONE tensor (no broadcast
loads).  rd/rdm (DVE not_equal, 2x) -> vertical-any; OR + horizontal 3-tap +
u8 convert on GPSIMD (off the critical DVE path); borders zeroed; one EARLY
u8 AllReduce(add) of the 256KB map overlaps the main loop.  bd>0 -> w=1+2b
built on GPSIMD while the loop runs.

Tail is pipelined per PSUM bank (4x512): ln(sums)-ln(gath) (exp/ln roundtrip
keeps every DVE operand 2-byte), *w, ones-matmul partition-reduce into the
freed sums bank row 0, one ACT copy w/ accum_out + 1/N scale, DMA out.

All bulk DMA rides SWDGE (gpsimd) - 16 engines; HWDGE only for tiny consts.
"""

import sys

sys.path.insert(0, "/opt/trn_rl_repo")

import numpy as np
import ml_dtypes

import concourse.bass as bass
import concourse.bacc as bacc
import concourse.tile as tile
from concourse import mybir
from concourse import bass_utils

F32 = mybir.dt.float32
BF16 = mybir.dt.bfloat16
U8 = mybir.dt.uint8
FP8 = mybir.dt.float8e4

C = 21            # channels
H = W = 512
NPIX = H * W      # 262144 pixels per core
P = 128           # SBUF partitions
CW = NPIX // P    # 2048 pixels per partition
XW = C * CW       # 43008 bytes per partition of fp8 x
T3W = CW + 1024   # 3072: t padded with +-512 halo
NCORES = 8
NTOT = float(NCORES * NPIX)
BANK = 512        # PSUM bank width in f32

Exp = mybir.ActivationFunctionType.Exp
Ln = mybir.ActivationFunctionType.Ln
Copy = mybir.ActivationFunctionType.Copy
op = mybir.AluOpType

# x DMA split points (channels): first piece small so exp starts early
X_SPLITS = [(0, 1), (1, 4), (4, 12), (12, 21)]


def build_nc(use_cc=True):
    nc = bacc.Bacc(
        "TRN2",
        target_bir_lowering=False,
        debug=False,
        num_devices=NCORES,
        num_swdge_queues=1,
        dynamic_dma_scratch_size=16384,
    )

    x_d = nc.dram_tensor("x", [P, XW], FP8, kind="ExternalInput")
    t3_d = nc.dram_tensor("t3", [P, T3W], BF16, kind="ExternalInput")
    out_d = nc.dram_tensor("out", [1, 1], F32, kind="ExternalOutput")

    ident_d = nc.inline_tensor(np.eye(P, dtype=ml_dtypes.bfloat16), name="ident")
    ones_d = nc.inline_tensor(np.ones((P, 1), ml_dtypes.bfloat16), name="ones")

    groups = [list(range(NCORES))]

    with tile.TileContext(nc) as tc:
        with (
            tc.tile_pool(name="singles", bufs=1) as singles,
            tc.tile_pool(name="expp", bufs=3) as expp,
            tc.tile_pool(name="mp", bufs=3) as mp,
            tc.tile_pool(name="tailp", bufs=2) as tailp,
            tc.tile_pool(name="psum", bufs=1, space="PSUM") as psum,
            tc.tile_pool(name="dram", bufs=1, space="DRAM") as dram,
        ):
            # tiny consts on HWDGE so the SWDGE queue starts with t3/x
            ident = singles.tile([P, P], BF16, tag="ident")
            ones = singles.tile([P, 1], BF16, tag="ones")
            nc.sync.dma_start(ident[:], ident_d[:])
            nc.sync.dma_start(ones[:], ones_d[:])

            # ---- loads: t3 first (boundary + masks need it), then x ----
            t3 = singles.tile([P, T3W], BF16, tag="t3")
            nc.sync.dma_start(t3[:], t3_d[:])
            x_t = singles.tile([P, XW], FP8, tag="x")
            for a, b in X_SPLITS:
                nc.gpsimd.dma_start(
                    x_t[:, a * CW : b * CW], x_d[:, a * CW : b * CW]
                )

            tshm = t3[:, 0:CW]            # flat t shifted -512 (row above)
            tden = t3[:, 512 : 512 + CW]  # flat t
            tsh = t3[:, 1024 : 1024 + CW]  # flat t shifted +512 (row below)

            # ---- boundary map ----
            # vertical any-diff on DVE (2x mode), the rest on GPSIMD so the
            # DVE queue is free for the mask ops.
            # All on DVE (cheap 2x-mode bf16 adds; masks are 0/1, `max` = OR,
            # keeping the map 0/1 for the AllReduce-max), finishing by ~10us so the AllReduce can launch
            # far ahead of when its result is needed.
            hp = tc.high_priority()
            hp.__enter__()
            rd = singles.tile([P, CW], BF16, tag="rd")
            nc.vector.tensor_tensor(rd[:], tden, tsh, op.not_equal)
            rdm = singles.tile([P, CW], BF16, tag="rdm")
            nc.vector.tensor_tensor(rdm[:], tshm, tden, op.not_equal)
            dv = singles.tile([P, CW], BF16, tag="dv")
            nc.vector.tensor_tensor(dv[:], rd[:], rdm[:], op.max)
            cat = singles.tile([P, CW], BF16, tag="cat")
            nc.vector.tensor_tensor(
                cat[:, 1 : CW - 1], dv[:, 0 : CW - 2], dv[:, 1 : CW - 1], op.max
            )
            ca8 = singles.tile([P, CW], U8, tag="ca8")
            nc.vector.tensor_tensor(
                ca8[:, 1 : CW - 1], cat[:, 1 : CW - 1], dv[:, 2:CW], op.max
            )
            cav = ca8[:].rearrange("P (r w) -> P r w", w=W)
            nc.vector.memset(cav[:, :, 0:1], 0)
            nc.vector.memset(cav[:, :, W - 1 : W], 0)
            nc.vector.memset(ca8[0:1, 0:W], 0)
            # engines can't address a start partition of 127; DMA a zero row
            zrow = singles.tile([1, W], U8, tag="zrow")
            nc.vector.memset(zrow[:], 0)
            nc.sync.dma_start(ca8[P - 1 : P, 3 * W : 4 * W], zrow[:])

            # cc path rides HWDGE (sync) so it never queues behind the bulk
            # x loads on the SWDGE FIFO; AllReduce(max) gives bd in {0,1}
            cc_in = dram.tile([P, CW], U8, tag="cc_in")
            cc_out = dram.tile([P, CW], U8, tag="cc_out")
            bd = singles.tile([P, CW], U8, tag="bd")
            w4 = singles.tile([P, CW], BF16, tag="w4")
            nc.sync.dma_start(cc_in[:], ca8[:])
            if use_cc:
                nc.gpsimd.collective_compute(
                    "AllReduce",
                    op.max,
                    replica_groups=groups,
                    ins=[cc_in.opt()],
                    outs=[cc_out.opt()],
                )
            else:
                cc_out = cc_in
            nc.sync.dma_start(bd[:], cc_out[:])
            hp.__exit__(None, None, None)

            # ---- main loop: channels in pairs (bigger ACT/DVE ops, fewer
            # semaphore hops); 21 = 10 pairs + 1 single ----
            sums = psum.tile([P, CW], F32, tag="sums")
            gath = psum.tile([P, CW], F32, tag="gath")
            for c0 in range(0, C, 2):
                nch = min(2, C - c0)
                fw = nch * CW
                ex = expp.tile([P, 2 * CW], BF16, tag="ex")
                nc.scalar.activation(
                    ex[:, 0:fw], x_t[:, c0 * CW : (c0 + nch) * CW], Exp
                )
                m = mp.tile([P, 2 * CW], BF16, tag="m")
                for k in range(nch):
                    nc.vector.tensor_scalar(
                        m[:, k * CW : (k + 1) * CW],
                        tden,
                        float(c0 + k),
                        None,
                        op.is_equal,
                    )
                mke = mp.tile([P, 2 * CW], BF16, tag="mke")
                nc.vector.tensor_tensor(
                    mke[:, 0:fw], m[:, 0:fw], ex[:, 0:fw], op.mult
                )
                for k in range(nch):
                    c = c0 + k
                    for j in range(4):
                        s = slice(j * BANK, (j + 1) * BANK)
                        sk = slice(k * CW + j * BANK, k * CW + (j + 1) * BANK)
                        nc.tensor.matmul(
                            sums[:, s],
                            ident[:],
                            ex[:, sk],
                            start=(c == 0),
                            stop=(c == C - 1),
                            skip_group_check=True,
                        )
                    for j in range(4):
                        s = slice(j * BANK, (j + 1) * BANK)
                        sk = slice(k * CW + j * BANK, k * CW + (j + 1) * BANK)
                        nc.tensor.matmul(
                            gath[:, s],
                            ident[:],
                            mke[:, sk],
                            start=(c == 0),
                            stop=(c == C - 1),
                            skip_group_check=True,
                        )

            # weights w = 1 + 2*bd (bd in {0,1} from AllReduce-max); gated by
            # the collective, which lands while the loop is still running
            nc.vector.tensor_scalar(w4[:], bd[:], 2.0, 1.0, op.mult, op.add)

            # ---- tail ----
            # All lns/subs first: none of these depend on the collective, so
            # they finish with the loop.  Only w4 -> wce -> fin -> copy -> out
            # sit behind bd, and the fin matmuls accumulate into one dead
            # gath bank (fresh accumulation group), not a live tile region.
            logs = singles.tile([P, CW], BF16, tag="logs")
            logg = singles.tile([P, CW], BF16, tag="logg")
            for j in range(4):
                s = slice(j * BANK, (j + 1) * BANK)
                nc.scalar.activation(logs[:, s], sums[:, s], Ln)
                nc.scalar.activation(logg[:, s], gath[:, s], Ln)
            ce = singles.tile([P, CW], BF16, tag="ce")
            for j in range(4):
                s = slice(j * BANK, (j + 1) * BANK)
                nc.vector.tensor_tensor(ce[:, s], logs[:, s], logg[:, s], op.subtract)
            wce = singles.tile([P, CW], BF16, tag="wce")
            for j in range(4):
                s = slice(j * BANK, (j + 1) * BANK)
                nc.vector.tensor_tensor(wce[:, s], ce[:, s], w4[:, s], op.mult)
            for j in range(4):
                s = slice(j * BANK, (j + 1) * BANK)
                nc.tensor.matmul(
                    gath[0:1, 0:BANK],
                    ones[:],
                    wce[:, s],
                    start=(j == 0),
                    stop=(j == 3),
                    skip_group_check=True,
                )

            scr = singles.tile([1, BANK], BF16, tag="scr")
            fin = singles.tile([1, 1], F32, tag="fin")
            nc.scalar.activation(
                scr[:], gath[0:1, 0:BANK], Copy, scale=1.0 / NTOT, accum_out=fin[:]
            )
            nc.gpsimd.dma_start(out_d[:], fin[:])

    nc.compile()
    return nc


_NC = None


def _get_nc():
    global _NC
    if _NC is None:
        _NC = build_nc()
    return _NC


def make_in_maps(inputs, targets):
    e4 = ml_dtypes.float8_e4m3
    in_maps = []
    for i in range(NCORES):
        x = np.asarray(inputs[i], dtype=np.float32).reshape(C, P, CW)
        x8 = np.ascontiguousarray(x.transpose(1, 0, 2)).astype(e4).reshape(P, XW)
        t = np.asarray(targets[i]).astype(np.uint8).reshape(-1)
        tp = np.zeros(NPIX + 1024, np.uint8)
        tp[512 : 512 + NPIX] = t
        t3 = np.lib.stride_tricks.as_strided(tp, (P, T3W), (CW, 1))
        t3 = np.ascontiguousarray(t3).astype(ml_dtypes.bfloat16)
        in_maps.append({"x": x8, "t3": t3})
    return in_maps


def run_device(inputs, targets, trace=False):
    nc = _get_nc()
    res = bass_utils.run_bass_kernel_spmd(
        nc,
        make_in_maps(inputs, targets),
        core_ids=list(range(NCORES)),
        trace=trace,
    )
    return res


def kernel(inputs, targets):
    res = run_device(inputs, targets, trace=False)
    # each core returns its local weighted-sum / (B*H*W); the global mean is
    # the sum of the 8 partials (final reduction of the batch shard).
    return np.float32(sum(float(r["out"][0, 0]) for r in res.results))
